# revision 6
# baseline (speedup 1.0000x reference)
"""Trainium2 Bass kernel for nn_Conv2dLayer_3195455668909.

Computes: conv_transpose2d(x, w, stride=2) -> 4x4 FIR (upfirdn2d) -> bias +
leaky-relu * sqrt(2) -> clamp(+-256), for x [8,512,64,64] f32,
weight [256,512,3,3], bias [256]. Output [8,256,128,128] f32.

Strategy (one batch image per NeuronCore, 8 cores):
 - Polyphase decomposition of the stride-2 transposed conv: 4 sub-convs on
   the 64x64 grid (2x2 / 2x1 / 1x2 / 1x1 taps), each as PE matmuls
   contracting over in-channels (bf16, fp32 PSUM accumulate).
 - FIR [1,3,3,1] x [1,3,3,1] = three 2-tap box filters per axis: 6 shifted
   tensor-adds on DVE over column-phase-separated row stacks.
 - Epilogue: leaky-relu + interleave on ACT, clamp on DVE, bf16 out,
   upcast to f32 on host.
All weight scaling (weight_gain, FIR normalization, act gain) is folded
into the weights/bias on the host.
"""
import math
from contextlib import ExitStack

import numpy as np
import ml_dtypes

import json

import concourse.bass as bass
import concourse.tile as tile
from concourse import bass2jax, mybir
from concourse.bass_utils import run_bass_kernel_spmd

N_CORES = 8
CI, CO, H, W = 512, 256, 64, 64
NIC, NOC = CI // 128, CO // 128   # channel chunks
XF = 66 * 66 + 8                  # padded-x flat length per channel (+slack)
NSLOT = 68                        # slots per fine row in a stack
NROW = 132                        # stack rows (fine row f -> stack row f+1)
LH = 131 * NSLOT                  # flat length for H-stage ops
CLAMP = 256.0
SLOPE = 0.2
ROWTAPS = {0: [(0, 0), (1, 2)], 1: [(0, 1)]}   # row-phase -> [(a', w_row)]
COLTAPS = {0: [(0, 0), (1, 2)], 1: [(0, 1)]}   # col-phase -> [(b', w_col)]
BF16 = mybir.dt.bfloat16
F32 = mybir.dt.float32


def _split_multi_waits(bir_bytes):
    """The walrus build here rejects instructions with more than one sync
    wait. Move extra waits onto same-engine NoOps inserted just before."""
    d = json.loads(bir_bytes)
    for fn in d["functions"]:
        for blk in fn["blocks"]:
            insts = blk.get("instructions")
            if not insts:
                continue
            out = []
            for ins in insts:
                si = ins.get("sync_info") or {}
                waits = si.get("on_wait") or []
                if len(waits) > 1:
                    for i, w in enumerate(waits[1:]):
                        out.append({
                            "debug": ins.get("debug", 0),
                            "engine": ins["engine"],
                            "ins": [],
                            "name": f"{ins['name']}-xw{i}",
                            "opcode": "NoOp",
                            "outs": [],
                            "sync_info": {"on_update": [], "on_wait": [w]},
                        })
                    si["on_wait"] = waits[:1]
                out.append(ins)
            blk["instructions"] = out
    return json.dumps(d).encode()


_orig_compile_bir_kernel = bass2jax.compile_bir_kernel


def _patched_compile_bir_kernel(ant_bir_str, *args, **kwargs):
    return _orig_compile_bir_kernel(_split_multi_waits(ant_bir_str), *args, **kwargs)


if bass2jax.compile_bir_kernel is not _patched_compile_bir_kernel:
    bass2jax.compile_bir_kernel = _patched_compile_bir_kernel


def _build_program():
    nc = bass.Bass()
    xp_d = nc.declare_dram_parameter("xp", [NIC, 128, XF], BF16, isOutput=False)
    wt_d = nc.declare_dram_parameter("wt", [NIC, 128, 3 * 3 * NOC * 128], BF16,
                                     isOutput=False)
    bs_d = nc.declare_dram_parameter("bs", [128, NOC], F32, isOutput=False)
    zo_d = nc.declare_dram_parameter("zo", [NOC, 2, 128, 64 * 128], BF16,
                                     isOutput=True)

    ctx = ExitStack()
    with ctx:
        tc = ctx.enter_context(tile.TileContext(nc))
        const = ctx.enter_context(tc.tile_pool(name="const", bufs=1))
        psum = ctx.enter_context(tc.tile_pool(name="psum", bufs=6, space="PSUM"))
        stks = ctx.enter_context(tc.tile_pool(name="stks", bufs=2))
        zp = ctx.enter_context(tc.tile_pool(name="zp", bufs=2))

        x_sb = const.tile([128, NIC, XF], BF16)
        w_sb = const.tile([128, NIC, 3, 3, NOC, 128], BF16)
        b_sb = const.tile([128, NOC], F32)
        for ic in range(NIC):
            nc.sync.dma_start(x_sb[:, ic], xp_d[ic])
            nc.sync.dma_start(
                w_sb[:, ic].rearrange("p a b o m -> p (a b o m)"), wt_d[ic]
            )
        nc.sync.dma_start(b_sb[:], bs_d[:])

        for oc in range(NOC):
            yE = stks.tile([128, NROW, NSLOT], BF16, tag="yE")
            yO = stks.tile([128, NROW, NSLOT], BF16, tag="yO")
            A = stks.tile([128, NROW, NSLOT], BF16, tag="A")
            nc.vector.memset(yE[:], 0.0)
            nc.vector.memset(yO[:], 0.0)
            stk = {0: yE, 1: yO}

            # --- conv: polyphase matmuls, accumulate taps x in-chunks ---
            for rp in (0, 1):
                nrows = 65 if rp == 0 else 64
                for cp in (0, 1):
                    taps = [(a_, wa, b_, wb)
                            for (a_, wa) in ROWTAPS[rp]
                            for (b_, wb) in COLTAPS[cp]]
                    for P0 in range(0, nrows, 7):
                        R = min(7, nrows - P0)
                        acc = psum.tile([128, R * 66], F32, tag="acc")
                        n = NIC * len(taps)
                        k = 0
                        for ic in range(NIC):
                            for (a_, wa, b_, wb) in taps:
                                start = (P0 + 1 - a_) * 66 + (1 - b_)
                                nc.tensor.matmul(
                                    acc[:],
                                    w_sb[:, ic, wa, wb, oc, :],
                                    x_sb[:, ic, start:start + R * 66],
                                    start=(k == 0), stop=(k == n - 1),
                                )
                                k += 1
                        r0 = 1 + rp + 2 * P0
                        nc.scalar.copy(
                            stk[cp][:, r0:r0 + 2 * R:2, 2:68],
                            acc[:].rearrange("p (r c) -> p r c", c=66),
                        )
            # zero the garbage cols of yO (phase cols Q=64,65 are invalid)
            nc.vector.memset(yO[:, :, 66:68], 0.0)

            yEf = yE[:].rearrange("p a b -> p (a b)")
            yOf = yO[:].rearrange("p a b -> p (a b)")
            Af = A[:].rearrange("p a b -> p (a b)")

            # --- H FIR: 3 box passes, col-phase separated ---
            def eop(dst, p, q):   # dst[s] = p[s] + q[s]
                nc.vector.tensor_add(dst[:, :LH], p[:, :LH], q[:, :LH])

            def oop(q, p):        # q[s] = q[s] + p[s+1]
                nc.vector.tensor_add(q[:, :LH], q[:, :LH], p[:, 1:LH + 1])

            eop(Af, yEf, yOf); oop(yOf, yEf)
            eop(yEf, Af, yOf); oop(yOf, Af)
            eop(Af, yEf, yOf); oop(yOf, yEf)
            # hE in A, hO in yO, scratch = yE

            # --- V FIR: 3 box passes, ping-pong (row shift = NSLOT elems) ---
            def vpass(dst, src, rows_out):
                m = rows_out * NSLOT
                nc.vector.tensor_add(
                    dst[:, :m], src[:, :m], src[:, NSLOT:m + NSLOT]
                )

            vpass(yEf, Af, 130); vpass(Af, yEf, 129); vpass(yEf, Af, 128)
            FE = yE   # z row t at stack row t; z[t,2T+1] = FE[t, T+2]
            vpass(Af, yOf, 130); vpass(yOf, Af, 129); vpass(Af, yOf, 128)
            FO = A    # z[t,2T] = FO[t, T+1]

            # --- epilogue: lrelu + interleave (ACT), clamp (DVE), DMA out ---
            for half in range(2):
                t0 = 64 * half
                Z = zp.tile([128, 64, 128], BF16, tag="Z")
                nc.scalar.activation(
                    Z[:, :, 0:128:2], FO[:, t0:t0 + 64, 1:65],
                    mybir.ActivationFunctionType.Identity,
                    bias=b_sb[:, oc:oc + 1], scale=1.0,
                )
                nc.scalar.activation(
                    Z[:, :, 1:128:2], FE[:, t0:t0 + 64, 2:66],
                    mybir.ActivationFunctionType.Identity,
                    bias=b_sb[:, oc:oc + 1], scale=1.0,
                )
                Zf = Z[:].rearrange("p a b -> p (a b)")
                # leaky relu: z = max(0.2*z, z), then clamp to +-256
                nc.vector.scalar_tensor_tensor(
                    Zf, Zf, SLOPE, Zf,
                    mybir.AluOpType.mult, mybir.AluOpType.max,
                )
                nc.vector.tensor_scalar(
                    Zf, Zf, CLAMP, -CLAMP,
                    mybir.AluOpType.min, mybir.AluOpType.max,
                )
                nc.sync.dma_start(zo_d[oc, half], Zf)
    return nc


def _prep_inputs(x, weight, bias):
    scale = math.sqrt(2.0) / (math.sqrt(CI * 9) * 16.0)
    w = (np.asarray(weight, np.float32) * scale)
    # [4 ic, 128 i, 3 a, 3 b, 2 oc, 128 o]
    wt = np.ascontiguousarray(
        w.reshape(NOC, 128, NIC, 128, 3, 3).transpose(2, 3, 4, 5, 0, 1)
    ).reshape(NIC, 128, 3 * 3 * NOC * 128).astype(ml_dtypes.bfloat16)
    b = (np.asarray(bias, np.float32) * math.sqrt(2.0)).reshape(NOC, 128)
    bs = np.ascontiguousarray(b.T).astype(np.float32)  # [128, NOC]
    xpad = np.zeros((N_CORES, CI, XF), np.float32)
    xpad[:, :, : 66 * 66] = np.pad(
        np.asarray(x, np.float32), [(0, 0), (0, 0), (1, 1), (1, 1)]
    ).reshape(N_CORES, CI, -1)
    xpad = xpad.reshape(N_CORES, NIC, 128, XF).astype(ml_dtypes.bfloat16)
    return xpad, wt, bs


def _run(x, weight, bias, trace=False, **kw):
    xpad, wt, bs = _prep_inputs(x, weight, bias)
    nc = _build_program()
    in_maps = [{"xp": xpad[c], "wt": wt, "bs": bs} for c in range(N_CORES)]
    res = run_bass_kernel_spmd(nc, in_maps, list(range(N_CORES)), trace=trace, **kw)
    outs = []
    for c in range(N_CORES):
        z = np.asarray(res.results[c]["zo"]).astype(np.float32)
        z = z.reshape(NOC, 2, 128, 64, 128)          # [oc, half, o, t, u]
        z = z.transpose(0, 2, 1, 3, 4).reshape(CO, 128, 128)
        outs.append(z)
    return np.stack(outs), res


def kernel(x, weight, bias):
    out, _ = _run(x, weight, bias, trace=False)
    return out
